# revision 38
# baseline (speedup 1.0000x reference)
"""Causal single-head attention on 8 Trainium2 NeuronCores.

Problem: x [32, 2048, 384] f32, Wq/Wk/Wv [384, 64] f32.
  q/k/v = x @ W;  out = softmax_causal(q k^T / sqrt(64)) @ v   -> [32, 2048, 64]

Strategy: data-parallel over batch (4 batches per core), weights replicated,
bf16 matmul path (fp32 accumulation), no collectives.

Per batch on one core (T=2048, C=384, H=64):
  - host pre-transposes x to xT [C, T] and casts to bf16
  - projection pass 1 with stationary [Wq|Wk] -> psum rows 0:64 = qT,
    rows 64:128 = kT ("hi" copy); pass 2 with [Wv|Wq] -> vT lo + qT hi.
    Per-strip SBUF->SBUF DMAs make the base-0 kT copy so the score
    matmuls can start as soon as the first projection strip is cast.
  - scores for pair p (s-chunks 2p, 2p+1) run as row-tiled K=64 matmul
    PAIRS into [128, 2, 512] psum strips; one exp ACTIVATE per strip
    (scale=1/8 fused) emits bf16 expT; only the two diagonal 128-blocks
    need masking -- sub-diagonal columns are never read by the
    fine-grained AV
  - expT tiles for all 8 pairs stay resident in SBUF; AV runs
    t-block-major (one outT[65, 512] PSUM bank at a time, ones-column
    stationary accumulates the softmax denominator in row 64), so the
    scores/exp pipeline gets 3 psum strip buffers and TensorE is not
    throttled by ScalarE's exp pace
  - AV matmuls start exactly at the diagonal t=128i
  - PE-transpose outT -> [t, 65], reciprocal of col 64, broadcast
    multiply, one DMA per 512-row block, f32 out
  - dummy matmuls at startup and batch boundaries keep the PE HAM clock
    gate at full rate
"""

import sys

sys.path.insert(0, "/opt/trn_rl_repo")

import numpy as np
import ml_dtypes

import concourse.bass as bass
import concourse.mybir as mybir
import concourse.tile as tile
from concourse import bacc
from concourse.bass_utils import run_bass_kernel_spmd

BF16 = mybir.dt.bfloat16
F32 = mybir.dt.float32
NP_BF16 = ml_dtypes.bfloat16

B, T_FULL, C, H = 32, 2048, 384, 64
N_CORES = 8
B_LOC = B // N_CORES
SCALE = float(H) ** -0.5
Exp = mybir.ActivationFunctionType.Exp
# Schraudolph-style exp for the DVE: y = int16(A*score + B) bitcast to bf16
# approximates exp(score/8) (max rel err ~4%, rms ~1.8%); used only on the
# strips farthest from the diagonal, where it costs ~1e-3 end-to-end.
LOG2E = float(np.log2(np.e))
DVE_A = 128.0 * LOG2E * SCALE
DVE_B = 127.0 * 128.0 - 7.5


def build_nc(b_loc=B_LOC, t=T_FULL):
    """Build the per-core Bass program (SPMD: same program on all cores)."""
    assert t % 512 == 0
    nc = bacc.Bacc(None, target_bir_lowering=False)
    cc = C // 128          # contraction chunks for projections
    ns = t // 128          # number of 128-wide s-chunks
    nt = t // 512          # number of 512-wide t-chunks
    npair = ns // 2        # s-chunk pairs (even chunk -> base 0, odd -> base 64)

    xT = nc.declare_dram_parameter("xT", [b_loc, C, t], BF16, isOutput=False)
    wqk_d = nc.declare_dram_parameter("Wqk", [C, 128], BF16, isOutput=False)
    wvq_d = nc.declare_dram_parameter("Wvq", [C, 128], BF16, isOutput=False)
    c16_d = nc.declare_dram_parameter("c16", [128, 2, 128], BF16, isOutput=False)
    id32_d = nc.declare_dram_parameter("ident32", [128, 128], F32, isOutput=False)
    outp = nc.declare_dram_parameter("out", [b_loc, t, H], F32, isOutput=True)

    with tile.TileContext(nc) as tc:
        with (
            tc.tile_pool(name="consts", bufs=1) as consts,
            tc.tile_pool(name="xt", bufs=2) as p_xt,
            tc.tile_pool(name="qk", bufs=2) as p_qk,
            tc.tile_pool(name="vv", bufs=2) as p_v,
            tc.tile_pool(name="exp", bufs=9) as p_exp,
            tc.tile_pool(name="oo", bufs=2) as p_o,
            tc.tile_pool(name="ps_big", bufs=3, space="PSUM") as ps_big,
            tc.tile_pool(name="ps_out", bufs=2, space="PSUM") as ps_out,
        ):
            # ---- PE warmup: dummy matmuls on a memset tile keep the HAM
            # clock gate open while the batch-0 xT DMA is in flight ----
            warm = consts.tile([128, 128], BF16)
            nc.gpsimd.memset(warm, 0.0)
            ps_w = ps_big.tile([128, 128], F32, tag="big", name="warmps")
            for w in range(56):
                nc.tensor.matmul(ps_w, warm, warm, start=True, stop=True)
            # ---- batch-0 xT prefetch: it is on the critical path ----
            xt0 = p_xt.tile([128, cc, t], BF16, tag="xt", name="xt0")
            half0 = t // 2
            nc.sync.dma_start(
                out=xt0[:, :, 0:half0],
                in_=xT[0, :, 0:half0].rearrange("(c p) t -> p c t", p=128),
            )
            # ---- constants (few, large DMAs: issue cost ~650ns each) ----
            wqk = consts.tile([128, cc, 128], BF16)
            nc.sync.dma_start(
                out=wqk, in_=wqk_d[:, :].rearrange("(c p) h -> p c h", p=128)
            )
            wvq = consts.tile([128, cc, 128], BF16)
            nc.sync.dma_start(
                out=wvq, in_=wvq_d[:, :].rearrange("(c p) h -> p c h", p=128)
            )
            c16 = consts.tile([128, 2, 128], BF16)
            nc.sync.dma_start(out=c16, in_=c16_d[:, :, :])
            ident16 = c16[:, 0, :]
            dmask = c16[:, 1, :]
            ident32 = consts.tile([128, 128], F32)
            nc.sync.dma_start(out=ident32, in_=id32_d[:, :])
            nc.sync.dma_start(
                out=xt0[:, :, half0:t],
                in_=xT[0, :, half0:t].rearrange("(c p) t -> p c t", p=128),
            )

            # ---------------- per-batch program ----------------
            def emit_p1(b):
                """Load xT, project q/k/v.
                Returns (g1, g2, vlo, klo, vaug): g1 rows 0:64 = qT lo, rows
                64:128 = kT hi; g2 rows 0:64 = vT, rows 64:128 = qT hi."""
                half = t // 2
                if b == 0:
                    xt_sb = xt0
                else:
                    xt_sb = p_xt.tile([128, cc, t], BF16, tag="xt", name=f"xt{b}")
                    nc.sync.dma_start(
                        out=xt_sb[:, :, 0:half],
                        in_=xT[b, :, 0:half].rearrange("(c p) t -> p c t", p=128),
                    )
                    nc.sync.dma_start(
                        out=xt_sb[:, :, half:t],
                        in_=xT[b, :, half:t].rearrange("(c p) t -> p c t", p=128),
                    )

                g1 = p_qk.tile([128, t], BF16, tag="g1", name=f"g1_{b}")
                g2 = p_qk.tile([128, t], BF16, tag="g2", name=f"g2_{b}")
                for tj in range(nt):
                    tr = slice(512 * tj, 512 * (tj + 1))
                    for w_sb, g_sb in ((wqk, g1), (wvq, g2)):
                        ps_g = ps_big.tile([128, 2, 512], F32, tag="big",
                                           name=f"psg{b}{tj}{w_sb is wvq}")
                        for c in range(cc):
                            nc.tensor.matmul(
                                ps_g[:, 0, :], w_sb[:, c, :], xt_sb[:, c, tr],
                                start=(c == 0), stop=(c == cc - 1),
                            )
                        if w_sb is wvq:
                            nc.scalar.copy(g_sb[:, tr], ps_g[:, 0, :])
                        else:
                            nc.vector.tensor_copy(g_sb[:, tr], ps_g[:, 0, :])
                # base-0 copy of kT, one DMA per 512-strip so the first
                # score matmuls can start as soon as strip 0 is cast
                klo = p_qk.tile([64, t], BF16, tag="klo", name=f"klo{b}")
                for tj in range(nt):
                    tr = slice(512 * tj, 512 * (tj + 1))
                    nc.sync.dma_start(out=klo[:, tr], in_=g1[64:128, tr])
                vlo = g2[0:64, :]
                # v tiles [s,H] + ones column via PE transpose of vT
                # (transposes emitted lazily per pair via emit_vtr)
                vaug = p_v.tile([128, ns, 65], BF16, tag="vaug", name=f"vaug{b}")
                nc.gpsimd.memset(vaug, 1.0)
                return g1, g2, vlo, klo, vaug

            def emit_vtr(b, i, vlo, vaug):
                """PE-transpose vT chunk i into vaug[:, i, 0:H]."""
                ps_tr = ps_big.tile([128, 64], BF16, tag="big",
                                    name=f"pstr{b}{i}")
                nc.tensor.transpose(
                    ps_tr, vlo[:, 128 * i:128 * (i + 1)], ident16[0:H, 0:H]
                )
                nc.vector.tensor_copy(vaug[:, i, 0:H], ps_tr)

            def emit_scores_pair(b, p, g1, g2, klo):
                """Score matmul pair + exp for s-chunks (2p, 2p+1) over
                t in [512*(p//2), t). Returns the bf16 expT tile [128, 2, t]."""
                i0, i1 = 2 * p, 2 * p + 1
                expT = p_exp.tile([128, 2, t], BF16, tag="expT", name=f"exp{b}{p}")
                # all strips 512-wide and 512-aligned; odd pairs overcompute
                # the sub-diagonal [256(p-1), 256p) which AV never reads
                strips = [(ts0, 512) for ts0 in range(512 * (p // 2), t, 512)]
                for sidx, (ts0, n) in enumerate(strips):
                    ps_s = ps_big.tile([128, 2, n], F32, tag="big",
                                       name=f"pss{b}{p}{ts0}")
                    nc.tensor.matmul(
                        ps_s[:, 0, :],
                        klo[:, 128 * i0:128 * (i0 + 1)],
                        g1[0:64, ts0:ts0 + n],
                        start=True, stop=True,
                    )
                    nc.tensor.matmul(
                        ps_s[:, 1, :],
                        g1[64:128, 128 * i1:128 * (i1 + 1)],
                        g2[64:128, ts0:ts0 + n],
                        start=True, stop=True,
                    )
                    if sidx == len(strips) - 1 and len(strips) >= 3:
                        # far strip: bit-trick exp on the DVE keeps the psum
                        # strip pool draining at twice the ScalarE-only pace
                        nc.vector.tensor_scalar(
                            out=expT[:, :, ts0:ts0 + n].bitcast(mybir.dt.int16),
                            in0=ps_s, scalar1=DVE_A, scalar2=DVE_B,
                            op0=mybir.AluOpType.mult, op1=mybir.AluOpType.add,
                        )
                    else:
                        off = 256 * (p % 2) if sidx == 0 else 0
                        nc.scalar.activation(
                            expT[:, :, ts0 + off:ts0 + n],
                            ps_s[:, :, off:n],
                            Exp, scale=SCALE,
                        )
                d0 = 256 * p
                nc.gpsimd.tensor_mul(
                    expT[:, 0, d0:d0 + 128], expT[:, 0, d0:d0 + 128], dmask
                )
                d1 = 256 * p + 128
                nc.gpsimd.tensor_mul(
                    expT[:, 1, d1:d1 + 128], expT[:, 1, d1:d1 + 128], dmask
                )
                return expT

            def emit_p3(b, j, outT_ps, o_all):
                """Transpose outT[65, 512] -> [t,65], normalize into o_all."""
                outTn = p_o.tile([65, 512], F32, tag="outTn", name=f"otn{b}{j}")
                nc.vector.tensor_copy(outTn, outT_ps)
                ps_o = ps_out.tile([128, 4, 65], F32, tag="outT", name=f"pso{b}{j}")
                for tt in range(4):
                    nc.tensor.transpose(
                        ps_o[:, tt, :],
                        outTn[:, 128 * tt:128 * (tt + 1)],
                        ident32[0:65, 0:65],
                    )
                zrec = p_o.tile([128, 4], F32, tag="zrec", bufs=4, name=f"zr{b}{j}")
                nc.vector.reciprocal(zrec, ps_o[:, :, H:H + 1])
                zbc = bass.AP(
                    tensor=zrec.tensor, offset=zrec.offset,
                    ap=[zrec.ap[0], zrec.ap[1], [0, H]],
                )
                nc.vector.tensor_tensor(
                    out=o_all[:, j], in0=ps_o[:, :, 0:H], in1=zbc,
                    op=mybir.AluOpType.mult,
                )
                dstj = outp[b, 512 * j:512 * (j + 1), :].rearrange(
                    "(tt tl) h -> tl tt h", tl=128
                )
                nc.sync.dma_start(out=dstj, in_=o_all[:, j])

            def emit_av_half(b, j, i_lo, i_hi, outT, expTs, vaug):
                """AV accumulation for t-block j over s-chunks [i_lo, i_hi];
                each chunk's matmul starts exactly at the diagonal."""
                last = 4 * j + 3
                for i in range(i_lo, i_hi + 1):
                    tlo = max(512 * j, 128 * i)
                    nc.tensor.matmul(
                        outT[:, tlo - 512 * j:],
                        vaug[:, i, :],
                        expTs[i // 2][:, i % 2, tlo:512 * (j + 1)],
                        start=(i == 0), stop=(i == last),
                    )

            for b in range(b_loc):
                if b > 0:
                    # keep the HAM clock gate open across the batch boundary
                    for w in range(4):
                        nc.tensor.matmul(ps_w, warm, warm, start=True, stop=True)
                g1, g2, vlo, klo, vaug = emit_p1(b)
                o_all = p_o.tile([128, nt, 4, H], F32, tag="o_all",
                                 name=f"oall{b}")
                expTs = []
                for p in range(npair):
                    emit_vtr(b, 2 * p, vlo, vaug)
                    emit_vtr(b, 2 * p + 1, vlo, vaug)
                    expTs.append(emit_scores_pair(b, p, g1, g2, klo))
                    if p % 2 == 1:
                        j = p // 2
                        outTj = ps_out.tile([65, 512], F32, tag="outT",
                                            name=f"outT{b}{j}")
                        emit_av_half(b, j, 0, 4 * j + 3, outTj, expTs, vaug)
                        emit_p3(b, j, outTj, o_all)

    nc.compile()
    return nc


def _shard_inputs(x, Wk, Wq, Wv, b_loc=B_LOC, t=T_FULL):
    ident32 = np.eye(128, dtype=np.float32)
    ident16 = ident32.astype(NP_BF16)
    mask = np.triu(np.ones((128, 128), dtype=np.float32)).astype(NP_BF16)
    c16 = np.ascontiguousarray(
        np.stack([ident16, mask], axis=1)
    )  # [128, 2, 128]
    wq16 = np.ascontiguousarray(Wq, dtype=np.float32).astype(NP_BF16)
    wk16 = np.ascontiguousarray(Wk, dtype=np.float32).astype(NP_BF16)
    wv16 = np.ascontiguousarray(Wv, dtype=np.float32).astype(NP_BF16)
    wqk = np.ascontiguousarray(np.concatenate([wq16, wk16], axis=1))
    wvq = np.ascontiguousarray(np.concatenate([wv16, wq16], axis=1))
    n_cores = x.shape[0] // b_loc
    xs = np.asarray(x, dtype=np.float32).reshape(n_cores, b_loc, t, C)
    in_maps = []
    for m in range(n_cores):
        xTm = np.ascontiguousarray(xs[m].transpose(0, 2, 1)).astype(NP_BF16)
        in_maps.append({
            "xT": xTm, "Wqk": wqk, "Wvq": wvq,
            "c16": c16, "ident32": ident32,
        })
    return in_maps


def _run(x, Wk, Wq, Wv, trace=False, **spmd_kwargs):
    nc = build_nc()
    in_maps = _shard_inputs(x, Wk, Wq, Wv)
    res = run_bass_kernel_spmd(
        nc, in_maps, core_ids=list(range(N_CORES)), trace=trace, **spmd_kwargs
    )
    out = np.concatenate([res.results[m]["out"] for m in range(N_CORES)], axis=0)
    return np.ascontiguousarray(out, dtype=np.float32), res


def kernel(x, Wk, Wq, Wv):
    out, _ = _run(x, Wk, Wq, Wv)
    return out


# revision 39
# speedup vs baseline: 1.0446x; 1.0446x over previous
"""Causal single-head attention on 8 Trainium2 NeuronCores.

Problem: x [32, 2048, 384] f32, Wq/Wk/Wv [384, 64] f32.
  q/k/v = x @ W;  out = softmax_causal(q k^T / sqrt(64)) @ v   -> [32, 2048, 64]

Strategy: data-parallel over batch (4 batches per core), weights replicated,
bf16 matmul path (fp32 accumulation), no collectives.

Per batch on one core (T=2048, C=384, H=64):
  - host pre-transposes x to xT [C, T] and casts to bf16
  - projection pass 1 with stationary [Wq|Wk] -> psum rows 0:64 = qT,
    rows 64:128 = kT ("hi" copy); pass 2 with [Wv|Wq] -> vT lo + qT hi.
    Per-strip SBUF->SBUF DMAs make the base-0 kT copy so the score
    matmuls can start as soon as the first projection strip is cast.
  - scores for pair p (s-chunks 2p, 2p+1) run as row-tiled K=64 matmul
    PAIRS into [128, 2, 512] psum strips; one exp ACTIVATE per strip
    (scale=1/8 fused) emits bf16 expT; only the two diagonal 128-blocks
    need masking -- sub-diagonal columns are never read by the
    fine-grained AV
  - expT tiles for all 8 pairs stay resident in SBUF; AV runs
    t-block-major (one outT[65, 512] PSUM bank at a time, ones-column
    stationary accumulates the softmax denominator in row 64), so the
    scores/exp pipeline gets 3 psum strip buffers and TensorE is not
    throttled by ScalarE's exp pace
  - AV matmuls start exactly at the diagonal t=128i
  - PE-transpose outT -> [t, 65], reciprocal of col 64, broadcast
    multiply, one DMA per 512-row block, f32 out
  - dummy matmuls at startup and batch boundaries keep the PE HAM clock
    gate at full rate
"""

import sys

sys.path.insert(0, "/opt/trn_rl_repo")

import numpy as np
import ml_dtypes

import concourse.bass as bass
import concourse.mybir as mybir
import concourse.tile as tile
from concourse import bacc
from concourse.bass_utils import run_bass_kernel_spmd

BF16 = mybir.dt.bfloat16
F32 = mybir.dt.float32
NP_BF16 = ml_dtypes.bfloat16

B, T_FULL, C, H = 32, 2048, 384, 64
N_CORES = 8
B_LOC = B // N_CORES
SCALE = float(H) ** -0.5
Exp = mybir.ActivationFunctionType.Exp
# Schraudolph-style exp for the DVE: y = int16(A*score + B) bitcast to bf16
# approximates exp(score/8) (max rel err ~4%, rms ~1.8%); used only on the
# strips farthest from the diagonal, where it costs ~1e-3 end-to-end.
LOG2E = float(np.log2(np.e))
DVE_A = 128.0 * LOG2E * SCALE
DVE_B = 127.0 * 128.0 - 7.5


def build_nc(b_loc=B_LOC, t=T_FULL):
    """Build the per-core Bass program (SPMD: same program on all cores)."""
    assert t % 512 == 0
    nc = bacc.Bacc(None, target_bir_lowering=False)
    cc = C // 128          # contraction chunks for projections
    ns = t // 128          # number of 128-wide s-chunks
    nt = t // 512          # number of 512-wide t-chunks
    npair = ns // 2        # s-chunk pairs (even chunk -> base 0, odd -> base 64)

    xT = nc.declare_dram_parameter("xT", [b_loc, C, t], BF16, isOutput=False)
    wqk_d = nc.declare_dram_parameter("Wqk", [C, 128], BF16, isOutput=False)
    wvq_d = nc.declare_dram_parameter("Wvq", [C, 128], BF16, isOutput=False)
    c16_d = nc.declare_dram_parameter("c16", [128, 2, 128], BF16, isOutput=False)
    id32_d = nc.declare_dram_parameter("ident32", [128, 128], F32, isOutput=False)
    outp = nc.declare_dram_parameter("out", [b_loc, t, H], F32, isOutput=True)

    with tile.TileContext(nc) as tc:
        with (
            tc.tile_pool(name="consts", bufs=1) as consts,
            tc.tile_pool(name="xt", bufs=2) as p_xt,
            tc.tile_pool(name="qk", bufs=2) as p_qk,
            tc.tile_pool(name="vv", bufs=2) as p_v,
            tc.tile_pool(name="exp", bufs=9) as p_exp,
            tc.tile_pool(name="oo", bufs=2) as p_o,
            tc.tile_pool(name="ps_big", bufs=3, space="PSUM") as ps_big,
            tc.tile_pool(name="ps_out", bufs=2, space="PSUM") as ps_out,
        ):
            # ---- PE warmup: dummy matmuls on a memset tile keep the HAM
            # clock gate open while the batch-0 xT DMA is in flight ----
            warm = consts.tile([128, 128], BF16)
            nc.gpsimd.memset(warm, 0.0)
            ps_w = ps_big.tile([128, 128], F32, tag="big", name="warmps")
            for w in range(56):
                nc.tensor.matmul(ps_w, warm, warm, start=True, stop=True)
            # ---- batch-0 xT prefetch: it is on the critical path ----
            xt0 = p_xt.tile([128, cc, t], BF16, tag="xt", name="xt0")
            half0 = t // 2
            nc.sync.dma_start(
                out=xt0[:, :, 0:half0],
                in_=xT[0, :, 0:half0].rearrange("(c p) t -> p c t", p=128),
            )
            # ---- constants (few, large DMAs: issue cost ~650ns each) ----
            wqk = consts.tile([128, cc, 128], BF16)
            nc.sync.dma_start(
                out=wqk, in_=wqk_d[:, :].rearrange("(c p) h -> p c h", p=128)
            )
            wvq = consts.tile([128, cc, 128], BF16)
            nc.sync.dma_start(
                out=wvq, in_=wvq_d[:, :].rearrange("(c p) h -> p c h", p=128)
            )
            c16 = consts.tile([128, 2, 128], BF16)
            nc.sync.dma_start(out=c16, in_=c16_d[:, :, :])
            ident16 = c16[:, 0, :]
            dmask = c16[:, 1, :]
            ident32 = consts.tile([128, 128], F32)
            nc.sync.dma_start(out=ident32, in_=id32_d[:, :])
            nc.sync.dma_start(
                out=xt0[:, :, half0:t],
                in_=xT[0, :, half0:t].rearrange("(c p) t -> p c t", p=128),
            )

            # ---------------- per-batch program ----------------
            def emit_p1(b):
                """Load xT, project q/k/v.
                Returns (g1, g2, vlo, klo, vaug): g1 rows 0:64 = qT lo, rows
                64:128 = kT hi; g2 rows 0:64 = vT, rows 64:128 = qT hi."""
                half = t // 2
                if b == 0:
                    xt_sb = xt0
                else:
                    xt_sb = p_xt.tile([128, cc, t], BF16, tag="xt", name=f"xt{b}")
                    nc.sync.dma_start(
                        out=xt_sb[:, :, 0:half],
                        in_=xT[b, :, 0:half].rearrange("(c p) t -> p c t", p=128),
                    )
                    nc.sync.dma_start(
                        out=xt_sb[:, :, half:t],
                        in_=xT[b, :, half:t].rearrange("(c p) t -> p c t", p=128),
                    )

                g1 = p_qk.tile([128, t], BF16, tag="g1", name=f"g1_{b}")
                g2 = p_qk.tile([128, t], BF16, tag="g2", name=f"g2_{b}")
                for tj in range(nt):
                    tr = slice(512 * tj, 512 * (tj + 1))
                    for w_sb, g_sb in ((wqk, g1), (wvq, g2)):
                        ps_g = ps_big.tile([128, 2, 512], F32, tag="big",
                                           name=f"psg{b}{tj}{w_sb is wvq}")
                        for c in range(cc):
                            nc.tensor.matmul(
                                ps_g[:, 0, :], w_sb[:, c, :], xt_sb[:, c, tr],
                                start=(c == 0), stop=(c == cc - 1),
                            )
                        if w_sb is wvq:
                            nc.scalar.copy(g_sb[:, tr], ps_g[:, 0, :])
                        else:
                            nc.vector.tensor_copy(g_sb[:, tr], ps_g[:, 0, :])
                # base-0 copy of kT, one DMA per 512-strip so the first
                # score matmuls can start as soon as strip 0 is cast
                klo = p_qk.tile([64, t], BF16, tag="klo", name=f"klo{b}")
                for tj in range(nt):
                    tr = slice(512 * tj, 512 * (tj + 1))
                    nc.sync.dma_start(out=klo[:, tr], in_=g1[64:128, tr])
                vlo = g2[0:64, :]
                # v tiles [s,H] + ones column via PE transpose of vT
                # (transposes emitted lazily per pair via emit_vtr)
                vaug = p_v.tile([128, ns, 65], BF16, tag="vaug", name=f"vaug{b}")
                nc.gpsimd.memset(vaug, 1.0)
                return g1, g2, vlo, klo, vaug

            def emit_vtr(b, i, vlo, vaug):
                """PE-transpose vT chunk i into vaug[:, i, 0:H]."""
                ps_tr = ps_big.tile([128, 64], BF16, tag="big",
                                    name=f"pstr{b}{i}")
                nc.tensor.transpose(
                    ps_tr, vlo[:, 128 * i:128 * (i + 1)], ident16[0:H, 0:H]
                )
                nc.vector.tensor_copy(vaug[:, i, 0:H], ps_tr)

            def emit_scores_pair(b, p, g1, g2, klo):
                """Score matmul pair + exp for s-chunks (2p, 2p+1) over
                t in [512*(p//2), t). Returns the bf16 expT tile [128, 2, t]."""
                i0, i1 = 2 * p, 2 * p + 1
                expT = p_exp.tile([128, 2, t], BF16, tag="expT", name=f"exp{b}{p}")
                # all strips 512-wide and 512-aligned; odd pairs overcompute
                # the sub-diagonal [256(p-1), 256p) which AV never reads
                strips = [(ts0, 512) for ts0 in range(512 * (p // 2), t, 512)]
                for sidx, (ts0, n) in enumerate(strips):
                    ps_s = ps_big.tile([128, 2, n], F32, tag="big",
                                       name=f"pss{b}{p}{ts0}")
                    nc.tensor.matmul(
                        ps_s[:, 0, :],
                        klo[:, 128 * i0:128 * (i0 + 1)],
                        g1[0:64, ts0:ts0 + n],
                        start=True, stop=True,
                    )
                    nc.tensor.matmul(
                        ps_s[:, 1, :],
                        g1[64:128, 128 * i1:128 * (i1 + 1)],
                        g2[64:128, ts0:ts0 + n],
                        start=True, stop=True,
                    )
                    if sidx == len(strips) - 1 and len(strips) >= 3:
                        # far strip: bit-trick exp on the DVE keeps the psum
                        # strip pool draining at twice the ScalarE-only pace
                        nc.vector.tensor_scalar(
                            out=expT[:, :, ts0:ts0 + n].bitcast(mybir.dt.int16),
                            in0=ps_s, scalar1=DVE_A, scalar2=DVE_B,
                            op0=mybir.AluOpType.mult, op1=mybir.AluOpType.add,
                        )
                    else:
                        off = 256 * (p % 2) if sidx == 0 else 0
                        nc.scalar.activation(
                            expT[:, :, ts0 + off:ts0 + n],
                            ps_s[:, :, off:n],
                            Exp, scale=SCALE,
                        )
                d0 = 256 * p
                nc.vector.tensor_mul(
                    expT[:, 0, d0:d0 + 128], expT[:, 0, d0:d0 + 128], dmask
                )
                d1 = 256 * p + 128
                nc.vector.tensor_mul(
                    expT[:, 1, d1:d1 + 128], expT[:, 1, d1:d1 + 128], dmask
                )
                return expT

            def emit_p3(b, j, outT_ps, o_all):
                """Transpose outT[65, 512] -> [t,65], normalize into o_all."""
                outTn = p_o.tile([65, 512], F32, tag="outTn", name=f"otn{b}{j}")
                nc.vector.tensor_copy(outTn, outT_ps)
                ps_o = ps_out.tile([128, 4, 65], F32, tag="outT", name=f"pso{b}{j}")
                for tt in range(4):
                    nc.tensor.transpose(
                        ps_o[:, tt, :],
                        outTn[:, 128 * tt:128 * (tt + 1)],
                        ident32[0:65, 0:65],
                    )
                zrec = p_o.tile([128, 4], F32, tag="zrec", bufs=4, name=f"zr{b}{j}")
                nc.vector.reciprocal(zrec, ps_o[:, :, H:H + 1])
                zbc = bass.AP(
                    tensor=zrec.tensor, offset=zrec.offset,
                    ap=[zrec.ap[0], zrec.ap[1], [0, H]],
                )
                nc.vector.tensor_tensor(
                    out=o_all[:, j], in0=ps_o[:, :, 0:H], in1=zbc,
                    op=mybir.AluOpType.mult,
                )
                dstj = outp[b, 512 * j:512 * (j + 1), :].rearrange(
                    "(tt tl) h -> tl tt h", tl=128
                )
                nc.sync.dma_start(out=dstj, in_=o_all[:, j])

            def emit_av_half(b, j, i_lo, i_hi, outT, expTs, vaug):
                """AV accumulation for t-block j over s-chunks [i_lo, i_hi];
                each chunk's matmul starts exactly at the diagonal."""
                last = 4 * j + 3
                for i in range(i_lo, i_hi + 1):
                    tlo = max(512 * j, 128 * i)
                    nc.tensor.matmul(
                        outT[:, tlo - 512 * j:],
                        vaug[:, i, :],
                        expTs[i // 2][:, i % 2, tlo:512 * (j + 1)],
                        start=(i == 0), stop=(i == last),
                    )

            for b in range(b_loc):
                if b > 0:
                    # keep the HAM clock gate open across the batch boundary
                    for w in range(4):
                        nc.tensor.matmul(ps_w, warm, warm, start=True, stop=True)
                g1, g2, vlo, klo, vaug = emit_p1(b)
                o_all = p_o.tile([128, nt, 4, H], F32, tag="o_all",
                                 name=f"oall{b}")
                expTs = []
                for p in range(npair):
                    emit_vtr(b, 2 * p, vlo, vaug)
                    emit_vtr(b, 2 * p + 1, vlo, vaug)
                    expTs.append(emit_scores_pair(b, p, g1, g2, klo))
                    if p % 2 == 1:
                        j = p // 2
                        outTj = ps_out.tile([65, 512], F32, tag="outT",
                                            name=f"outT{b}{j}")
                        emit_av_half(b, j, 0, 4 * j + 3, outTj, expTs, vaug)
                        emit_p3(b, j, outTj, o_all)

    nc.compile()
    return nc


def _shard_inputs(x, Wk, Wq, Wv, b_loc=B_LOC, t=T_FULL):
    ident32 = np.eye(128, dtype=np.float32)
    ident16 = ident32.astype(NP_BF16)
    mask = np.triu(np.ones((128, 128), dtype=np.float32)).astype(NP_BF16)
    c16 = np.ascontiguousarray(
        np.stack([ident16, mask], axis=1)
    )  # [128, 2, 128]
    wq16 = np.ascontiguousarray(Wq, dtype=np.float32).astype(NP_BF16)
    wk16 = np.ascontiguousarray(Wk, dtype=np.float32).astype(NP_BF16)
    wv16 = np.ascontiguousarray(Wv, dtype=np.float32).astype(NP_BF16)
    wqk = np.ascontiguousarray(np.concatenate([wq16, wk16], axis=1))
    wvq = np.ascontiguousarray(np.concatenate([wv16, wq16], axis=1))
    n_cores = x.shape[0] // b_loc
    xs = np.asarray(x, dtype=np.float32).reshape(n_cores, b_loc, t, C)
    in_maps = []
    for m in range(n_cores):
        xTm = np.ascontiguousarray(xs[m].transpose(0, 2, 1)).astype(NP_BF16)
        in_maps.append({
            "xT": xTm, "Wqk": wqk, "Wvq": wvq,
            "c16": c16, "ident32": ident32,
        })
    return in_maps


def _run(x, Wk, Wq, Wv, trace=False, **spmd_kwargs):
    nc = build_nc()
    in_maps = _shard_inputs(x, Wk, Wq, Wv)
    res = run_bass_kernel_spmd(
        nc, in_maps, core_ids=list(range(N_CORES)), trace=trace, **spmd_kwargs
    )
    out = np.concatenate([res.results[m]["out"] for m in range(N_CORES)], axis=0)
    return np.ascontiguousarray(out, dtype=np.float32), res


def kernel(x, Wk, Wq, Wv):
    out, _ = _run(x, Wk, Wq, Wv)
    return out
